# revision 6
# baseline (speedup 1.0000x reference)
"""Expert-parallel MoE MLP (BaseMLPExperts) for 8 TRN2 NeuronCores.

Reference computation (per expert e):
    y[:, e, :] = gelu_exact(x[:, e, :] @ wi[e]) @ wo[e]
with T=8192 tokens, E=8 experts, H=1024 hidden, I=4096 intermediate, fp32.

Sharding: expert-parallel - core e owns expert e (its x slice, wi[e], wo[e]).
No cross-core communication.

Per-core device kernel, all-bf16 on the PE (fp32 PSUM accumulation, exact-erf
GELU on fp32 psum, fp32 output; measured rel-err ~3e-3 end to end, gate 2e-2):

Single fused phase, h1 never leaves SBUF. Both weight matrices are
SBUF-resident in bf16 (8 MB each), plus one 4 MB h1 tile per 512-token block:
  for each 512-token tile:
    GEMM1: psum[i128, t512] = sum_ho wi[ho, io].T @ xT[ho, t] (8 MMs, N=512);
           ACT applies exact GELU on psum eviction -> h1[i128, io, t512] bf16.
    GEMM2: for each 128-token sub-block, accumulate over all 32 io-tiles:
           psum[t128, h512] += h1[:, io, sub].T @ wo[io, h-half] (64 MMs);
           DVE evicts to an SBUF f32 tile, DMA to y.
Program order on the PE serializes G1(tt) -> G2(tt) -> G1(tt+1), so a single
h1 buffer has no WAR hazard and the MM stream is gapless: every dependency
(gelu of an io-tile, wo/wi/xt DMA) completes thousands of MM-slots before its
consuming MM issues.

bf16 everywhere halves DMA and makes LDWEIGHTS eligible for Fast Weight Load
(~97 ns vs ~187 ns for 4-byte f32r), which is what lets the MM issue gap reach
the warm roofline of N/2.4GHz + 2.5 ns.

DMA: 32 MB in (x 16, wi 8, wo 8), 32 MB out (y f32) - no h1 scratch traffic.
Priming order puts wi columns 0:128 first on the SP queue so the first MM
only waits for ~1.25 MB; wo streams under G1(0)/G2(0) long before its first
use at ~74 us.
"""

import numpy as np

import concourse.bass as bass
import concourse.mybir as mybir
import concourse.tile as tile
from concourse import bacc
from concourse.bass_utils import run_bass_kernel_spmd

T, E, H, I = 8192, 8, 1024, 4096
P = 128
F32 = mybir.dt.float32
BF16 = mybir.dt.bfloat16

TT = 512             # token tile
NT = T // TT         # 16
HT = H // P          # 8 k-tiles for GEMM1
IT = I // P          # 32 i-tiles
SUB = TT // P        # 4 sub-blocks per tile in GEMM2

# run_bass_kernel_spmd kwargs injected by test harness (e.g. trace=True)
RUN_KWARGS: dict = {}
LAST_RESULT = None

_NC = None


def _build():
    nc = bacc.Bacc("TRN2", target_bir_lowering=False, debug=False, num_devices=8)

    xT = nc.dram_tensor("xT", [H, T], BF16, kind="ExternalInput").ap()
    wi = nc.dram_tensor("wi", [H, I], BF16, kind="ExternalInput").ap()
    wo = nc.dram_tensor("wo", [I, H], BF16, kind="ExternalInput").ap()
    y = nc.dram_tensor("y", [T, H], F32, kind="ExternalOutput").ap()

    xT_r = xT.rearrange("(ho p) t -> p ho t", p=P)      # [128, 8, T]
    wi_r = wi.rearrange("(ho p) i -> p ho i", p=P)      # [128, 8, I]
    wo_r = wo.rearrange("(io p) h -> p io h", p=P)      # [128, 32, H]

    with tile.TileContext(nc) as tc:
        wpool = tc.alloc_tile_pool(name="wpool", bufs=1)
        wis = wpool.tile([P, HT, I], BF16, name="wis")      # 64 KB/part
        wos = wpool.tile([P, IT, H], BF16, name="wos")      # 64 KB/part
        h1p = tc.alloc_tile_pool(name="h1p", bufs=1)
        h1 = h1p.tile([P, IT, TT], BF16, name="h1")         # 32 KB/part

        with (
            tc.tile_pool(name="xt_pool", bufs=3) as xt_pool,
            tc.tile_pool(name="yo_pool", bufs=3) as yo_pool,
            tc.tile_pool(name="ps1_pool", bufs=4, space="PSUM") as ps1_pool,
            tc.tile_pool(name="ps2_pool", bufs=4, space="PSUM") as ps2_pool,
        ):
            def load_xt(tt, engs):
                t0 = tt * TT
                xt = xt_pool.tile([P, HT, TT], BF16, name="xt", tag="xt")
                for q in range(4):
                    engs[q % 2].dma_start(
                        out=xt[:, 2 * q : 2 * q + 2, :],
                        in_=xT_r[:, 2 * q : 2 * q + 2, t0 : t0 + TT],
                    )
                return xt

            # ---- priming (emission order == per-queue DMA order) ----
            # SP: wi cols 0:128 (unblocks io-group 0) + half of xt0.
            nc.sync.dma_start(out=wis[:, :, 0:P], in_=wi_r[:, :, 0:P])
            xt0 = load_xt(0, (nc.scalar, nc.sync))
            # Pool/GpSimd queue warms up latest: io-groups 1..3.
            nc.gpsimd.dma_start(out=wis[:, :, P:512], in_=wi_r[:, :, P:512])
            xt1 = load_xt(1, (nc.scalar, nc.sync))
            # wi cols 512:4096 in 1MB pieces, alternating SP/ACT; piece p is
            # needed ~(18 + 6.9p) us in, lands well before that.
            engs2 = [nc.sync, nc.scalar]
            for pc in range(1, 8):
                engs2[pc % 2].dma_start(
                    out=wis[:, :, pc * 512 : (pc + 1) * 512],
                    in_=wi_r[:, :, pc * 512 : (pc + 1) * 512],
                )
            # wo: first use at G2(0) ~74 us in, all 32 io-tiles resident by
            # ~65 us: io 0:16 on the GpSimd queue, the rest split SP/ACT
            # behind the wi pieces.
            for k in range(4):
                nc.gpsimd.dma_start(
                    out=wos[:, 4 * k : 4 * k + 4, :],
                    in_=wo_r[:, 4 * k : 4 * k + 4, :],
                )
            for k in range(4, 8):
                engs2[k % 2].dma_start(
                    out=wos[:, 4 * k : 4 * k + 4, :],
                    in_=wo_r[:, 4 * k : 4 * k + 4, :],
                )

            def g1(tt, xt):
                for io in range(IT):
                    ps = ps1_pool.tile([P, TT], F32, name="ps1", tag="ps1")
                    for ho in range(HT):
                        nc.tensor.matmul(
                            ps[:],
                            wis[:, ho, io * P : (io + 1) * P],
                            xt[:, ho, :],
                            start=(ho == 0),
                            stop=(ho == HT - 1),
                        )
                    nc.scalar.activation(
                        h1[:, io, :], ps[:], mybir.ActivationFunctionType.Gelu
                    )

            def g2(tt):
                for s in range(SUB):
                    pss = [
                        ps2_pool.tile([P, 512], F32, name="ps2", tag="ps2")
                        for _ in range(2)
                    ]
                    for io in range(IT):
                        st = h1[:, io, s * P : (s + 1) * P]
                        for hh in range(2):
                            nc.tensor.matmul(
                                pss[hh],
                                st,
                                wos[:, io, hh * 512 : (hh + 1) * 512],
                                start=(io == 0),
                                stop=(io == IT - 1),
                            )
                    yo = yo_pool.tile([P, H], F32, name="yo", tag="yo")
                    last = tt == NT - 1 and s == SUB - 1
                    for hh in range(2):
                        # tail: evict the final psum pair on two engines
                        if last and hh == 0:
                            nc.scalar.activation(
                                yo[:, 0:512],
                                pss[0][:],
                                mybir.ActivationFunctionType.Identity,
                            )
                        else:
                            nc.vector.tensor_copy(
                                yo[:, hh * 512 : (hh + 1) * 512], pss[hh][:]
                            )
                    t0 = tt * TT + s * P
                    if last:
                        for g, eng in ((0, nc.scalar), (1, nc.gpsimd)):
                            eng.dma_start(
                                out=y[t0 + 64 * g : t0 + 64 * (g + 1), :],
                                in_=yo[64 * g : 64 * (g + 1), :],
                            )
                    else:
                        eng = (nc.scalar, nc.gpsimd)[(tt * SUB + s) % 2]
                        eng.dma_start(out=y[t0 : t0 + P, :], in_=yo[:])

            xt_cur, xt_nxt = xt0, xt1
            for tt in range(NT):
                g1(tt, xt_cur)
                if tt + 2 < NT:
                    xt_new = load_xt(tt + 2, (nc.sync, nc.sync))
                else:
                    xt_new = None
                g2(tt)
                xt_cur, xt_nxt = xt_nxt, xt_new

        h1p.release()
        wpool.release()

    nc.compile()
    return nc


def kernel(x: np.ndarray, wi: np.ndarray, wo: np.ndarray) -> np.ndarray:
    global _NC, LAST_RESULT
    x = np.asarray(x, dtype=np.float32)
    wi = np.asarray(wi, dtype=np.float32)
    wo = np.asarray(wo, dtype=np.float32)
    assert x.shape == (T, E, H) and wi.shape == (E, H, I) and wo.shape == (E, I, H)

    if _NC is None:
        _NC = _build()

    import ml_dtypes

    bf16 = ml_dtypes.bfloat16
    in_maps = [
        {
            "xT": np.ascontiguousarray(x[:, e, :].T.astype(bf16)),
            "wi": np.ascontiguousarray(wi[e].astype(bf16)),
            "wo": np.ascontiguousarray(wo[e].astype(bf16)),
        }
        for e in range(E)
    ]
    try:
        res = run_bass_kernel_spmd(
            _NC, in_maps, core_ids=list(range(E)), **RUN_KWARGS
        )
    except Exception:
        res = run_bass_kernel_spmd(
            _NC, in_maps, core_ids=list(range(E)), **RUN_KWARGS
        )
    LAST_RESULT = res
    out = np.stack([res.results[e]["y"] for e in range(E)], axis=1)
    return np.ascontiguousarray(out.astype(np.float32, copy=False))


# revision 8
# speedup vs baseline: 1.0109x; 1.0109x over previous
"""Expert-parallel MoE MLP (BaseMLPExperts) for 8 TRN2 NeuronCores.

Reference computation (per expert e):
    y[:, e, :] = gelu_exact(x[:, e, :] @ wi[e]) @ wo[e]
with T=8192 tokens, E=8 experts, H=1024 hidden, I=4096 intermediate, fp32.

Sharding: expert-parallel - core e owns expert e (its x slice, wi[e], wo[e]).
No cross-core communication.

Per-core device kernel, all-bf16 on the PE (fp32 PSUM accumulation, exact-erf
GELU on fp32 psum, fp32 output; measured rel-err ~3e-3 end to end, gate 2e-2):

Single fused phase, h1 never leaves SBUF. Both weight matrices are
SBUF-resident in bf16 (8 MB each), plus one 4 MB h1 tile per 512-token block:
  for each 512-token tile:
    GEMM1: psum[i128, t512] = sum_ho wi[ho, io].T @ xT[ho, t] (8 MMs, N=512);
           ACT applies exact GELU on psum eviction -> h1[i128, io, t512] bf16.
    GEMM2: for each 128-token sub-block, accumulate over all 32 io-tiles:
           psum[t128, h512] += h1[:, io, sub].T @ wo[io, h-half] (64 MMs);
           DVE evicts to an SBUF f32 tile, DMA to y.
Program order on the PE serializes G1(tt) -> G2(tt) -> G1(tt+1), so a single
h1 buffer has no WAR hazard and the MM stream is gapless: every dependency
(gelu of an io-tile, wo/wi/xt DMA) completes thousands of MM-slots before its
consuming MM issues.

bf16 everywhere halves DMA and makes LDWEIGHTS eligible for Fast Weight Load
(~97 ns vs ~187 ns for 4-byte f32r), which is what lets the MM issue gap reach
the warm roofline of N/2.4GHz + 2.5 ns.

DMA: 32 MB in (x 16, wi 8, wo 8), 32 MB out (y f32) - no h1 scratch traffic.
Priming order puts wi columns 0:128 first on the SP queue so the first MM
only waits for ~1.25 MB; wo streams under G1(0)/G2(0) long before its first
use at ~74 us.
"""

import numpy as np

import concourse.bass as bass
import concourse.mybir as mybir
import concourse.tile as tile
from concourse import bacc
from concourse.bass_utils import run_bass_kernel_spmd

T, E, H, I = 8192, 8, 1024, 4096
P = 128
F32 = mybir.dt.float32
BF16 = mybir.dt.bfloat16

TT = 512             # token tile
NT = T // TT         # 16
HT = H // P          # 8 k-tiles for GEMM1
IT = I // P          # 32 i-tiles
SUB = TT // P        # 4 sub-blocks per tile in GEMM2

# run_bass_kernel_spmd kwargs injected by test harness (e.g. trace=True)
RUN_KWARGS: dict = {}
LAST_RESULT = None

_NC = None


def _build():
    nc = bacc.Bacc("TRN2", target_bir_lowering=False, debug=False, num_devices=8)

    xT = nc.dram_tensor("xT", [H, T], BF16, kind="ExternalInput").ap()
    wi = nc.dram_tensor("wi", [H, I], BF16, kind="ExternalInput").ap()
    wo = nc.dram_tensor("wo", [I, H], BF16, kind="ExternalInput").ap()
    y = nc.dram_tensor("y", [T, H], F32, kind="ExternalOutput").ap()

    xT_r = xT.rearrange("(ho p) t -> p ho t", p=P)      # [128, 8, T]
    wi_r = wi.rearrange("(ho p) i -> p ho i", p=P)      # [128, 8, I]
    wo_r = wo.rearrange("(io p) h -> p io h", p=P)      # [128, 32, H]

    with tile.TileContext(nc) as tc:
        wpool = tc.alloc_tile_pool(name="wpool", bufs=1)
        wis = wpool.tile([P, HT, I], BF16, name="wis")      # 64 KB/part
        wos = wpool.tile([P, IT, H], BF16, name="wos")      # 64 KB/part
        h1p = tc.alloc_tile_pool(name="h1p", bufs=1)
        h1 = h1p.tile([P, IT, TT], BF16, name="h1")         # 32 KB/part

        with (
            tc.tile_pool(name="xt_pool", bufs=3) as xt_pool,
            tc.tile_pool(name="yo_pool", bufs=3) as yo_pool,
            tc.tile_pool(name="ps1_pool", bufs=4, space="PSUM") as ps1_pool,
            tc.tile_pool(name="ps2_pool", bufs=4, space="PSUM") as ps2_pool,
        ):
            def load_xt(tt, engs):
                t0 = tt * TT
                xt = xt_pool.tile([P, HT, TT], BF16, name="xt", tag="xt")
                for q in range(4):
                    engs[q % 2].dma_start(
                        out=xt[:, 2 * q : 2 * q + 2, :],
                        in_=xT_r[:, 2 * q : 2 * q + 2, t0 : t0 + TT],
                    )
                return xt

            # ---- priming (emission order == per-queue DMA order) ----
            # A dma_start OCCUPIES its issuing engine for the whole transfer,
            # so ACT (gelu evictions -> psum-ring liveness) and DVE (psum
            # copies) must issue no loads: SP carries wi+xt, GpSimd wo+y.
            # SP: wi cols 0:128 first (unblocks io-group 0 with ~1.25 MB).
            nc.sync.dma_start(out=wis[:, :, 0:P], in_=wi_r[:, :, 0:P])
            xt0 = load_xt(0, (nc.sync, nc.gpsimd))
            # io-groups 1..3 on GpSimd (its queue warms up a bit later).
            nc.gpsimd.dma_start(out=wis[:, :, P:512], in_=wi_r[:, :, P:512])
            # wi cols 512:4096 in 1MB pieces on SP; piece p is needed
            # ~(12 + 6.9p) us in and lands a few us ahead of that.
            for pc in range(1, 8):
                nc.sync.dma_start(
                    out=wis[:, :, pc * 512 : (pc + 1) * 512],
                    in_=wi_r[:, :, pc * 512 : (pc + 1) * 512],
                )
            xt1 = load_xt(1, (nc.sync, nc.sync))
            # wo: first use at G2(0) ~70 us in, consumed io-major over
            # ~14 us/sub-block; io 0:20 on GpSimd, io 20:32 behind SP's wi.
            for k in range(5):
                nc.gpsimd.dma_start(
                    out=wos[:, 4 * k : 4 * k + 4, :],
                    in_=wo_r[:, 4 * k : 4 * k + 4, :],
                )
            for k in range(5, 8):
                nc.sync.dma_start(
                    out=wos[:, 4 * k : 4 * k + 4, :],
                    in_=wo_r[:, 4 * k : 4 * k + 4, :],
                )

            def g1(tt, xt):
                for io in range(IT):
                    ps = ps1_pool.tile([P, TT], F32, name="ps1", tag="ps1")
                    for ho in range(HT):
                        nc.tensor.matmul(
                            ps[:],
                            wis[:, ho, io * P : (io + 1) * P],
                            xt[:, ho, :],
                            start=(ho == 0),
                            stop=(ho == HT - 1),
                        )
                    nc.scalar.activation(
                        h1[:, io, :], ps[:], mybir.ActivationFunctionType.Gelu
                    )

            def g2(tt):
                for s in range(SUB):
                    pss = [
                        ps2_pool.tile([P, 512], F32, name="ps2", tag="ps2")
                        for _ in range(2)
                    ]
                    for io in range(IT):
                        st = h1[:, io, s * P : (s + 1) * P]
                        for hh in range(2):
                            nc.tensor.matmul(
                                pss[hh],
                                st,
                                wos[:, io, hh * 512 : (hh + 1) * 512],
                                start=(io == 0),
                                stop=(io == IT - 1),
                            )
                    yo = yo_pool.tile([P, H], F32, name="yo", tag="yo")
                    last = tt == NT - 1 and s == SUB - 1
                    for hh in range(2):
                        # tail: evict the final psum pair on two engines
                        if last and hh == 0:
                            nc.scalar.activation(
                                yo[:, 0:512],
                                pss[0][:],
                                mybir.ActivationFunctionType.Identity,
                            )
                        else:
                            nc.vector.tensor_copy(
                                yo[:, hh * 512 : (hh + 1) * 512], pss[hh][:]
                            )
                    t0 = tt * TT + s * P
                    if last:
                        for g, eng in ((0, nc.sync), (1, nc.gpsimd)):
                            eng.dma_start(
                                out=y[t0 + 64 * g : t0 + 64 * (g + 1), :],
                                in_=yo[64 * g : 64 * (g + 1), :],
                            )
                    else:
                        nc.gpsimd.dma_start(out=y[t0 : t0 + P, :], in_=yo[:])

            xt_cur, xt_nxt = xt0, xt1
            for tt in range(NT):
                g1(tt, xt_cur)
                if tt + 2 < NT:
                    xt_new = load_xt(tt + 2, (nc.sync, nc.sync))
                else:
                    xt_new = None
                g2(tt)
                xt_cur, xt_nxt = xt_nxt, xt_new

        h1p.release()
        wpool.release()

    nc.compile()
    return nc


def kernel(x: np.ndarray, wi: np.ndarray, wo: np.ndarray) -> np.ndarray:
    global _NC, LAST_RESULT
    x = np.asarray(x, dtype=np.float32)
    wi = np.asarray(wi, dtype=np.float32)
    wo = np.asarray(wo, dtype=np.float32)
    assert x.shape == (T, E, H) and wi.shape == (E, H, I) and wo.shape == (E, I, H)

    if _NC is None:
        _NC = _build()

    import ml_dtypes

    bf16 = ml_dtypes.bfloat16
    in_maps = [
        {
            "xT": np.ascontiguousarray(x[:, e, :].T.astype(bf16)),
            "wi": np.ascontiguousarray(wi[e].astype(bf16)),
            "wo": np.ascontiguousarray(wo[e].astype(bf16)),
        }
        for e in range(E)
    ]
    try:
        res = run_bass_kernel_spmd(
            _NC, in_maps, core_ids=list(range(E)), **RUN_KWARGS
        )
    except Exception:
        res = run_bass_kernel_spmd(
            _NC, in_maps, core_ids=list(range(E)), **RUN_KWARGS
        )
    LAST_RESULT = res
    out = np.stack([res.results[e]["y"] for e in range(E)], axis=1)
    return np.ascontiguousarray(out.astype(np.float32, copy=False))
